# revision 26
# baseline (speedup 1.0000x reference)
"""Trainium2 Bass kernel for nn_Block_21251498181165 (gnn_message_passing).

Strategy: the radial envelope relu(1 - |r|^2/4) is exactly zero for any edge
with wrapped distance >= 2, and every output term is linear in the radial
encoding, so those edges contribute exactly 0.  Launch A computes |r|^2 for
all 320k edges on device (dma_gather of pos rows, sharded 40k edges/core).
The host then compacts the surviving edge list (index bookkeeping only),
partitions nodes contiguously across the 8 cores (graph parallel, scatter
stays local to the src partition), and Launch B does the full message
passing for the survivors: per-128-edge tile it gathers pos/x_a/x_v/x_d
rows, computes the geometry + all seven rank-decomposed tensor products on
device, folds the ACE embedding (B terms) into the same per-edge payload,
and segment-sums into the node table with a one-hot matmul.
"""

import numpy as np

N_NODES, N_EDGES = 20000, 320000
NC = 8
EPC = N_EDGES // NC          # 40000 edges per core (launch A)
GRP = (EPC + 127) // 128     # 313 groups of 128
EPAD = GRP * 128             # 40064
NPC = N_NODES // NC          # 2500 nodes per core
T_DEF = (NPC + 127) // 128   # 20 aligned 128-node windows per core
OUTC = 304                   # 64 + 96 + 144 output cols per node

M_ORDER = ["000", "110", "011", "101", "112", "211", "202"]

_CACHE = {}


def _f32(x):
    return np.ascontiguousarray(x, dtype=np.float32)


# ---------------------------------------------------------------- launch A


def _build_a():
    import concourse.bacc as bacc
    import concourse.mybir as mybir
    import concourse.tile as tile

    nc = bacc.Bacc("TRN2", target_bir_lowering=False, debug=False, num_devices=NC)
    f32, i16 = mybir.dt.float32, mybir.dt.int16
    pospad = nc.dram_tensor("pospad", [N_NODES, 64], f32, kind="ExternalInput")
    gsrc = nc.dram_tensor("gsrc", [128, GRP * 8], i16, kind="ExternalInput")
    gdst = nc.dram_tensor("gdst", [128, GRP * 8], i16, kind="ExternalInput")
    xsq = nc.dram_tensor("xsq", [128, GRP], f32, kind="ExternalOutput")

    Alu = mybir.AluOpType
    chunks = []
    g0 = 0
    while g0 < GRP:
        gc = min(80, GRP - g0)
        chunks.append((g0, gc))
        g0 += gc

    with tile.TileContext(nc) as tc:
        with tc.tile_pool(name="cst", bufs=1) as cst, \
             tc.tile_pool(name="wk", bufs=2) as wk:
            gi_s = cst.tile([128, GRP * 8], i16)
            nc.sync.dma_start(out=gi_s[:], in_=gsrc.ap())
            gi_d = cst.tile([128, GRP * 8], i16)
            nc.sync.dma_start(out=gi_d[:], in_=gdst.ap())

            for gs, gc in chunks:
                ni = gc * 128
                hi = gs * 128 + ni
                nreg = ni - (hi - EPC if hi > EPC else 0)
                gts = wk.tile([128, gc, 64], f32, tag="gts")
                if gs * 128 + gc * 128 > EPC:
                    nc.gpsimd.memset(gts[:], 0.0)
                nc.gpsimd.dma_gather(gts[:], pospad.ap(),
                                     gi_s[:, gs * 8:(gs + gc) * 8],
                                     ni, nreg, 64, elem_step=64,
                                     single_packet=False)
                gtd = wk.tile([128, gc, 64], f32, tag="gtd")
                if gs * 128 + gc * 128 > EPC:
                    nc.gpsimd.memset(gtd[:], 0.0)
                nc.gpsimd.dma_gather(gtd[:], pospad.ap(),
                                     gi_d[:, gs * 8:(gs + gc) * 8],
                                     ni, nreg, 64, elem_step=64,
                                     single_packet=False)
                d = wk.tile([128, gc, 3], f32, tag="d")
                nc.vector.tensor_sub(d[:], gtd[:, :, 0:3], gts[:, :, 0:3])
                t1 = wk.tile([128, gc, 3], f32, tag="t1")
                nc.vector.tensor_scalar(t1[:], d[:], 10.0, None, op0=Alu.is_gt)
                d2 = wk.tile([128, gc, 3], f32, tag="d2")
                nc.vector.scalar_tensor_tensor(d2[:], t1[:], -20.0, d[:],
                                               op0=Alu.mult, op1=Alu.add)
                t2 = wk.tile([128, gc, 3], f32, tag="t2")
                nc.vector.tensor_scalar(t2[:], d2[:], -10.0, None, op0=Alu.is_lt)
                d3 = wk.tile([128, gc, 3], f32, tag="d3")
                nc.vector.scalar_tensor_tensor(d3[:], t2[:], 20.0, d2[:],
                                               op0=Alu.mult, op1=Alu.add)
                sq = wk.tile([128, gc, 3], f32, tag="sq")
                nc.vector.tensor_mul(sq[:], d3[:], d3[:])
                xs = wk.tile([128, gc], f32, tag="xs")
                nc.vector.reduce_sum(xs[:], sq[:], axis=mybir.AxisListType.X)
                nc.sync.dma_start(out=xsq.ap()[:, gs:gs + gc], in_=xs[:])

    nc.compile()
    return nc


# ---------------------------------------------------------------- launch B


def _build_b(T):
    import concourse.bacc as bacc
    import concourse.bass as bass
    import concourse.mybir as mybir
    import concourse.tile as tile
    from concourse.masks import make_identity

    SUB = 2
    W = 128 * SUB
    assert T % SUB == 0
    NMAC = T // SUB
    nc = bacc.Bacc("TRN2", target_bir_lowering=False, debug=False, num_devices=NC)
    f32, i32 = mybir.dt.float32, mybir.dt.int32
    Alu = mybir.AluOpType
    Act = mybir.ActivationFunctionType
    IOA = bass.IndirectOffsetOnAxis

    def din(name, shape, dt=f32):
        return nc.dram_tensor(name, shape, dt, kind="ExternalInput")

    pos3 = din("pos3", [N_NODES + 1, 3])
    posx_d = din("posx", [N_NODES + 1, 307])
    srcg = din("srcg", [128, T], i32)
    dstg = din("dstg", [128, T], i32)
    elocf = din("elocf", [128, T])
    khalf4_d = din("khalf4", [128, 8 * SUB])
    w2sa_d = din("w2sa", [8, 128])
    w2sb_d = din("w2sb", [8, 96])
    wavd_d = din("wavd", [8, 112])
    wx1_d = din("wx1", [64, 48])
    wv1_d = din("wv1", [32, 80])
    wd1_d = din("wd1", [16, 48])
    woa_d = din("woa", [16, 128])
    wov_d = din("wov", [16, 96])
    wod_d = din("wod", [16, 32])
    rep16_d = din("rep16", [3, 80])
    rep144a_d = din("rep144a", [9, 128])
    rep144b_d = din("rep144b", [9, 128])
    rep144c_d = din("rep144c", [9, 32])
    rep32_d = din("rep32", [3, 96])
    repv32_d = din("repv32", [32, 96])
    outp = nc.dram_tensor("outp", [T * 128, OUTC], f32, kind="ExternalOutput")

    with tile.TileContext(nc) as tc:
        with tc.tile_pool(name="cst", bufs=1) as cst, \
             tc.tile_pool(name="wk", bufs=2) as wk, \
             tc.tile_pool(name="wkb", bufs=1) as wkb, \
             tc.tile_pool(name="tp", bufs=3, space="PSUM") as tp, \
             tc.tile_pool(name="pacc", bufs=1, space="PSUM") as pacc, \
             tc.tile_pool(name="pdqp", bufs=2, space="PSUM") as pdqp, \
             tc.tile_pool(name="pnb", bufs=1, space="PSUM") as pnb:

            def cload(dram, shape, dt=f32):
                t = cst.tile(shape, dt, tag=f"c_{dram.name}")
                nc.sync.dma_start(out=t[:], in_=dram.ap())
                return t

            ident = cst.tile([128, 128], f32)
            make_identity(nc, ident[:])
            iota_i = cst.tile([128, 128], i32)
            nc.gpsimd.iota(iota_i[:], pattern=[[1, 128]], base=0,
                           channel_multiplier=0)
            iota_f = cst.tile([128, 128], f32)
            nc.vector.tensor_copy(out=iota_f[:], in_=iota_i[:])
            b_eps = cst.tile([128, 1], f32)
            nc.gpsimd.memset(b_eps[:], 1e-12)

            sg_s = cload(srcg, [128, T], i32)
            dg_s = cload(dstg, [128, T], i32)
            el_s = cload(elocf, [128, T])
            khalf4 = cload(khalf4_d, [128, 8 * SUB])
            w2sa = cload(w2sa_d, [8, 128])
            w2sb = cload(w2sb_d, [8, 96])
            wavd = cload(wavd_d, [8, 112])
            wx1 = cload(wx1_d, [64, 48])
            wv1 = cload(wv1_d, [32, 80])
            wd1 = cload(wd1_d, [16, 48])
            woa = cload(woa_d, [16, 128])
            wov = cload(wov_d, [16, 96])
            wod = cload(wod_d, [16, 32])
            rep16 = cload(rep16_d, [3, 80])
            rep144a = cload(rep144a_d, [9, 128])
            rep144b = cload(rep144b_d, [9, 128])
            rep144c = cload(rep144c_d, [9, 32])
            rep32 = cload(rep32_d, [3, 96])
            repv32 = cload(repv32_d, [32, 96])

            for m in range(NMAC):
                ts0 = SUB * m
                # --- gathers (batched per macro)
                psB = wk.tile([128, SUB, 3], f32, tag="psB")
                dxB = wk.tile([128, SUB, 307], f32, tag="dxB")
                for tt in range(SUB):
                    t = ts0 + tt
                    nc.gpsimd.indirect_dma_start(
                        out=psB[:, tt, :], out_offset=None, in_=pos3.ap(),
                        in_offset=IOA(ap=sg_s[:, t:t + 1], axis=0))
                    nc.gpsimd.indirect_dma_start(
                        out=dxB[:, tt, :], out_offset=None, in_=posx_d.ap(),
                        in_offset=IOA(ap=dg_s[:, t:t + 1], axis=0))
                pdB = dxB[:, :, 0:3]
                xaB = dxB[:, :, 3:67]
                xvB = dxB[:, :, 67:163]
                xdB = dxB[:, :, 163:307]

                # --- geometry on [128, 4, *]
                d = wk.tile([128, SUB, 3], f32, tag="d")
                nc.vector.tensor_sub(d[:], pdB, psB[:])
                t1 = wk.tile([128, SUB, 3], f32, tag="t1")
                nc.vector.tensor_scalar(t1[:], d[:], 10.0, None, op0=Alu.is_gt)
                d2 = wk.tile([128, SUB, 3], f32, tag="d2")
                nc.vector.scalar_tensor_tensor(d2[:], t1[:], -20.0, d[:],
                                               op0=Alu.mult, op1=Alu.add)
                t2 = wk.tile([128, SUB, 3], f32, tag="t2")
                nc.vector.tensor_scalar(t2[:], d2[:], -10.0, None, op0=Alu.is_lt)
                r = wk.tile([128, SUB, 3], f32, tag="r")
                nc.vector.scalar_tensor_tensor(r[:], t2[:], 20.0, d2[:],
                                               op0=Alu.mult, op1=Alu.add)
                sq = wk.tile([128, SUB, 3], f32, tag="sq")
                nc.vector.tensor_mul(sq[:], r[:], r[:])
                xs = wk.tile([128, SUB], f32, tag="xs")
                nc.vector.reduce_sum(xs[:], sq[:], axis=mybir.AxisListType.X)
                s = wk.tile([128, SUB], f32, tag="s")
                nc.scalar.activation(s[:], xs[:], Act.Sqrt,
                                     bias=b_eps[:, 0:1], scale=0.25)
                env = wk.tile([128, SUB], f32, tag="env")
                nc.scalar.activation(env[:], xs[:], Act.Relu, bias=1.0,
                                     scale=-0.25)
                s_b = s[:].unsqueeze(2)
                q = wk.tile([128, SUB, 8], f32, tag="q")
                nc.vector.tensor_mul(q[:], khalf4[:].rearrange(
                    "p (s k) -> p s k", k=8), s_b.to_broadcast([128, SUB, 8]))
                nc.vector.tensor_scalar_add(q[:], q[:], 0.25)
                n_i = wk.tile([128, SUB, 8], i32, tag="n_i")
                nc.vector.tensor_copy(out=n_i[:], in_=q[:])
                n_f = wk.tile([128, SUB, 8], f32, tag="n_f")
                nc.vector.tensor_copy(out=n_f[:], in_=n_i[:])
                dfr = wk.tile([128, SUB, 8], f32, tag="dfr")
                nc.vector.tensor_sub(dfr[:], q[:], n_f[:])
                tg = wk.tile([128, SUB, 8], f32, tag="tg")
                nc.vector.tensor_scalar(tg[:], dfr[:], 0.5, None, op0=Alu.is_gt)
                dq = wk.tile([128, SUB, 8], f32, tag="dq")
                nc.vector.scalar_tensor_tensor(dq[:], tg[:], -1.0, dfr[:],
                                               op0=Alu.mult, op1=Alu.add)
                radc = wk.tile([128, SUB, 8], f32, tag="radc")
                nc.scalar.activation(radc[:], dq[:], Act.Sin, bias=0.0,
                                     scale=float(2 * np.pi))
                rad = wk.tile([128, SUB, 8], f32, tag="rad")
                nc.vector.tensor_mul(rad[:], radc[:],
                                     env[:].unsqueeze(2).to_broadcast([128, SUB, 8]))
                u = wk.tile([128, SUB, 3], f32, tag="u")
                nc.vector.tensor_scalar_mul(u[:], r[:], 4.25)
                usq = wk.tile([128, SUB, 3], f32, tag="usq")
                nc.vector.tensor_mul(usq[:], u[:], u[:])
                nsq = wk.tile([128, SUB], f32, tag="nsq")
                nc.vector.reduce_sum(nsq[:], usq[:], axis=mybir.AxisListType.X)
                nrm = wk.tile([128, SUB], f32, tag="nrm")
                nc.scalar.activation(nrm[:], nsq[:], Act.Sqrt, bias=b_eps[:, 0:1])
                sg = wk.tile([128, SUB], f32, tag="sg")
                nc.scalar.activation(sg[:], nrm[:], Act.Sigmoid)
                rhat = wk.tile([128, SUB, 3], f32, tag="rhat")
                nc.vector.tensor_mul(rhat[:], u[:],
                                     sg[:].unsqueeze(2).to_broadcast([128, SUB, 3]))
                rr = wk.tile([128, SUB, 9], f32, tag="rr")
                for i in range(3):
                    nc.vector.tensor_mul(rr[:, :, 3 * i:3 * i + 3], rhat[:],
                                         rhat[:, :, i:i + 1].to_broadcast([128, SUB, 3]))

                # --- transposed layout [feat, 512]; 4 transposes share a psum tile
                cp_flip = [0]

                def psum_copy(dst, srcp):
                    cp_flip[0] ^= 1
                    if cp_flip[0]:
                        nc.vector.tensor_copy(out=dst, in_=srcp)
                    else:
                        nc.scalar.activation(dst, srcp, Act.Copy)

                def transB(src_fn, rows, tag):
                    pt = tp.tile([rows, W], f32, tag="tp")
                    for tt in range(SUB):
                        nc.tensor.transpose(out=pt[:, 128 * tt:128 * (tt + 1)],
                                            in_=src_fn(tt), identity=ident[:])
                    st = wkb.tile([rows, W], f32, tag=tag)
                    psum_copy(st[:], pt[:])
                    return st

                radT = transB(lambda tt: rad[:, tt, :], 8, "radT")
                rhatT = transB(lambda tt: rhat[:, tt, :], 3, "rhatT")
                rrT = transB(lambda tt: rr[:, tt, :], 9, "rrT")
                xaT = transB(lambda tt: xaB[:, tt, :], 64, "xaT")
                xvT = [transB(lambda tt, _x=x: xvB[:, tt, :].rearrange(
                    "p (c x) -> p c x", x=3)[:, :, _x], 32, f"xvT{x}")
                    for x in range(3)]
                xdT = [transB(lambda tt, _ij=ij: xdB[:, tt, :].rearrange(
                    "p (c ij) -> p c ij", ij=9)[:, :, _ij], 16, f"xdT{ij}")
                    for ij in range(9)]

                def proj_split(lhsT, rhs_ap, mm, blocks, tagp):
                    pt = tp.tile([mm, W], f32, tag="tp")
                    nc.tensor.matmul(pt[:], lhsT=lhsT, rhs=rhs_ap, start=True,
                                     stop=True)
                    out = []
                    for bi, (off, rows) in enumerate(blocks):
                        st = wkb.tile([rows, W], f32, tag=f"{tagp}{bi}")
                        psum_copy(st[:], pt[off:off + rows, :])
                        out.append(st)
                    return out

                B4 = [(32 * i, 16) for i in range(4)]
                B3 = [(32 * i, 16) for i in range(3)]
                B2 = [(0, 16), (32, 16)]
                s_t = (proj_split(w2sa[:], radT[:], 128, B4, "s_ta") +
                       proj_split(w2sb[:], radT[:], 96, B3, "s_tb"))
                Ra, Rv, Rd = proj_split(wavd[:], radT[:], 112,
                                        [(0, 64), (64, 32), (96, 16)], "Ravd")
                a000, a011 = proj_split(wx1[:], xaT[:], 48, B2, "axt")
                # xvT rows are (x*32+c); project each 32-row x-block
                av = [proj_split(wv1[:], xvT[x][:], 80, B3, f"av{x}")
                      for x in range(3)]
                ad = [proj_split(wd1[:], xdT[ij][:], 48, B2, f"ad{ij}")
                      for ij in range(9)]
                u16 = proj_split(rep16[:], rhatT[:], 80, B3, "u16")
                u32 = proj_split(rep32[:], rhatT[:], 96, [(0, 96)], "u32")[0]
                u144 = (proj_split(rep144a[:], rrT[:], 128, B4, "u144") +
                        proj_split(rep144b[:], rrT[:], 128, B4, "u144y") +
                        proj_split(rep144c[:], rrT[:], 32, [(0, 16)], "u144x"))

                def sl(mn):
                    return s_t[M_ORDER.index(mn)][:]

                def u16s(x):
                    return u16[x][:]

                def u144s(ij):
                    return u144[ij][:]

                Ra, Rv, Rd = Ra[:], Rv[:], Rd[:]
                a000, a011 = a000[:], a011[:]

                # --- psi_a
                pa = pacc.tile([64, W], f32, tag="pa")
                c000 = wk.tile([16, W], f32, tag="c000")
                nc.vector.tensor_mul(c000[:], a000, sl("000"))
                nc.tensor.matmul(pa[:], lhsT=woa[:, 0:64], rhs=c000[:],
                                 start=True, stop=False)
                acc = wk.tile([16, W], f32, tag="acc110")
                tmp = wk.tile([16, W], f32, tag="tmp110")
                nc.vector.tensor_mul(acc[:], av[0][0][:], u16s(0))
                for x in (1, 2):
                    nc.vector.tensor_mul(tmp[:], av[x][0][:], u16s(x))
                    nc.vector.tensor_add(acc[:], acc[:], tmp[:])
                c110 = wk.tile([16, W], f32, tag="c110")
                nc.vector.tensor_mul(c110[:], acc[:], sl("110"))
                nc.tensor.matmul(pa[:], lhsT=woa[:, 64:128], rhs=c110[:],
                                 start=False, stop=True)
                ta = wk.tile([64, W], f32, tag="ta")
                nc.vector.tensor_add(ta[:], pa[:], Ra)

                # --- psi_v (blocks xyz*32+cv; complete each block's group)
                pv = pacc.tile([96, W], f32, tag="pv")
                c011p = wk.tile([16, W], f32, tag="c011p")
                nc.vector.tensor_mul(c011p[:], a011, sl("011"))
                cxt = wk.tile([16, W], f32, tag="cxt")
                for x in range(3):
                    blk = pv[32 * x:32 * x + 32, :]
                    nc.vector.tensor_mul(cxt[:], c011p[:], u16s(x))
                    nc.tensor.matmul(blk, lhsT=wov[:, 0:32], rhs=cxt[:],
                                     start=True, stop=False)
                    nc.vector.tensor_mul(cxt[:], av[x][1][:], sl("101"))
                    nc.tensor.matmul(blk, lhsT=wov[:, 32:64], rhs=cxt[:],
                                     start=False, stop=False)
                    nc.vector.tensor_mul(acc[:], ad[3 * x][0][:], u16s(0))
                    for j in (1, 2):
                        nc.vector.tensor_mul(tmp[:], ad[3 * x + j][0][:], u16s(j))
                        nc.vector.tensor_add(acc[:], acc[:], tmp[:])
                    nc.vector.tensor_mul(cxt[:], acc[:], sl("211"))
                    nc.tensor.matmul(blk, lhsT=wov[:, 64:96], rhs=cxt[:],
                                     start=False, stop=True)
                vrep = tp.tile([96, W], f32, tag="tp")
                nc.tensor.matmul(vrep[:], lhsT=repv32[:], rhs=Rv, start=True,
                                 stop=True)
                phv = wk.tile([96, W], f32, tag="phv")
                nc.vector.tensor_mul(phv[:], vrep[:], u32[:])
                tv = wk.tile([96, W], f32, tag="tv")
                nc.vector.tensor_add(tv[:], pv[:], phv[:])

                # --- psi_d (per-ij accumulate + fold B term)
                a112s = []
                for x in range(3):
                    a1x = wk.tile([16, W], f32, tag=f"a112s{x}")
                    nc.vector.tensor_mul(a1x[:], av[x][2][:], sl("112"))
                    a112s.append(a1x)
                tds = []
                phd = wk.tile([16, W], f32, tag="phd")
                for ij in range(9):
                    i, j = divmod(ij, 3)
                    pq = pdqp.tile([16, W], f32, tag="pdq")
                    nc.vector.tensor_mul(cxt[:], a112s[i][:], u16s(j))
                    nc.tensor.matmul(pq[:], lhsT=wod[:, 0:16], rhs=cxt[:],
                                     start=True, stop=False)
                    cx2 = wk.tile([16, W], f32, tag="cx2")
                    nc.gpsimd.tensor_mul(cx2[:], ad[ij][1][:], sl("202"))
                    nc.tensor.matmul(pq[:], lhsT=wod[:, 16:32], rhs=cx2[:],
                                     start=False, stop=True)
                    nc.gpsimd.tensor_mul(phd[:], Rd, u144s(ij))
                    td = wk.tile([16, W], f32, tag=f"td{ij}")
                    nc.vector.tensor_add(td[:], pq[:], phd[:])
                    tds.append(td)

                # --- back to edge layout + scatter, per subtile
                payload = wk.tile([128, SUB, OUTC], f32, tag="payload")
                pba = tp.tile([128, 64 * SUB], f32, tag="tp")
                for tt in range(SUB):
                    nc.tensor.transpose(out=pba[:, 64 * tt:64 * (tt + 1)],
                                        in_=ta[:, 128 * tt:128 * (tt + 1)],
                                        identity=ident[0:64, 0:64])
                nc.vector.tensor_copy(
                    out=payload[:, :, 0:64],
                    in_=pba[:].rearrange("p (t c) -> p t c", t=SUB))
                pbv = tp.tile([128, 96 * SUB], f32, tag="tp")
                for tt in range(SUB):
                    nc.tensor.transpose(out=pbv[:, 96 * tt:96 * (tt + 1)],
                                        in_=tv[:, 128 * tt:128 * (tt + 1)],
                                        identity=ident[0:96, 0:96])
                pbv4 = pbv[:].rearrange("p (t x c) -> p t x c",
                                        t=SUB, x=3).transpose([0, 1, 3, 2])
                nc.vector.tensor_copy(
                    out=payload[:, :, 64:160].rearrange(
                        "p t (c x) -> p t c x", x=3),
                    in_=pbv4)
                for tt in range(SUB):
                    pbd = tp.tile([128, 144], f32, tag="tp")
                    for ij in range(9):
                        nc.tensor.transpose(
                            out=pbd[:, 16 * ij:16 * (ij + 1)],
                            in_=tds[ij][:, 128 * tt:128 * (tt + 1)],
                            identity=ident[0:16, 0:16])
                    nc.vector.tensor_copy(
                        out=payload[:, tt, 160:304].rearrange(
                            "p (c ij) -> p c ij", ij=9),
                        in_=pbd[:].rearrange("p (ij c) -> p ij c",
                                             ij=9).transpose([0, 2, 1]))

                for tt in range(SUB):
                    t = ts0 + tt
                    oh = wk.tile([128, 128], f32, tag="oh")
                    nc.vector.tensor_tensor(
                        out=oh[:],
                        in0=el_s[:, t:t + 1].to_broadcast([128, 128]),
                        in1=iota_f[:], op=Alu.is_equal)
                    nb = pnb.tile([128, OUTC], f32, tag="nb")
                    nc.tensor.matmul(nb[:], lhsT=oh[:], rhs=payload[:, tt, :],
                                     start=True, stop=True)
                    nbs = wk.tile([128, OUTC], f32, tag="nbs")
                    nc.scalar.activation(nbs[:], nb[:], Act.Copy)
                    nc.sync.dma_start(out=outp.ap()[128 * t:128 * (t + 1), :],
                                      in_=nbs[:])

    nc.compile()
    return nc


# ---------------------------------------------------------------- host side


def _wrap_idx(idx):
    """int16 wrapped layout for dma_gather: [128, EPAD//16], idx i at
    [i%16, i//16], replicated across the 8 groups of 16 partitions."""
    w = np.full(EPAD, -1, np.int16)
    w[:len(idx)] = idx.astype(np.int16)
    w16 = w.reshape(-1, 16).T
    return np.ascontiguousarray(np.tile(w16, (8, 1)))


def _host_prep_b(inputs, mask, T):
    pos = _f32(inputs["pos"])
    src = np.asarray(inputs["src"]).astype(np.int64)
    dst = np.asarray(inputs["dst"]).astype(np.int64)
    surv = np.nonzero(mask)[0]
    ss, dd = src[surv], dst[surv]
    order = np.argsort(ss, kind="stable")
    ss, dd = ss[order], dd[order]
    core = ss // NPC
    loc = ss % NPC
    win = loc // 128
    el = (loc % 128).astype(np.float32)

    per_core = []
    for c in range(NC):
        m = core == c
        cs, cd_, cw, ce = ss[m], dd[m], win[m], el[m]
        srcg = np.full((T, 128), N_NODES, np.int32)
        dstg = np.zeros((T, 128), np.int32)
        elocf = np.zeros((T, 128), np.float32)
        tile_map = []
        tpos = 0
        for w in range(T_DEF):
            wm = cw == w
            k = int(wm.sum())
            idx0 = 0
            while True:
                take = min(128, k - idx0) if k > idx0 else 0
                if tpos >= T:
                    raise RuntimeError("tile overflow; rebuild with larger T")
                sel = np.nonzero(wm)[0][idx0:idx0 + take]
                srcg[tpos, :take] = cs[sel]
                dstg[tpos, :take] = cd_[sel]
                elocf[tpos, :take] = ce[sel]
                tile_map.append(w)
                tpos += 1
                idx0 += take
                if idx0 >= k:
                    break
        while tpos < T:
            tile_map.append(T_DEF - 1)
            tpos += 1
        per_core.append((np.ascontiguousarray(srcg.T),
                         np.ascontiguousarray(dstg.T),
                         np.ascontiguousarray(elocf.T), tile_map))
    return per_core


def _weights_b(inputs):
    g = lambda n: np.asarray(inputs[n], np.float32)
    w2 = [g(f"w{m}_2").T for m in M_ORDER]          # each [8, 16]
    w2sa = np.zeros((8, 128), np.float32)
    w2sb = np.zeros((8, 96), np.float32)
    for i in range(4):
        w2sa[:, 32 * i:32 * i + 16] = w2[i]
    for i in range(3):
        w2sb[:, 32 * i:32 * i + 16] = w2[4 + i]
    wavd = _f32(np.concatenate([g("ace_wa"), g("ace_wv"), g("ace_wd")], 0).T)
    wx1 = np.zeros((64, 48), np.float32)
    wx1[:, 0:16] = g("w000_1").T
    wx1[:, 32:48] = g("w011_1").T
    wv1 = np.zeros((32, 80), np.float32)
    wv1[:, 0:16] = g("w110_1").T
    wv1[:, 32:48] = g("w101_1").T
    wv1[:, 64:80] = g("w112_1").T
    wd1 = np.zeros((16, 48), np.float32)
    wd1[:, 0:16] = g("w211_1").T
    wd1[:, 32:48] = g("w202_1").T
    woa = _f32(np.concatenate([g("w000_o").T, g("w110_o").T], 1))
    wov = _f32(np.concatenate([g("w011_o").T, g("w101_o").T, g("w211_o").T], 1))
    wod = _f32(np.concatenate([g("w112_o").T, g("w202_o").T], 1))
    rep16 = np.zeros((3, 80), np.float32)
    for x in range(3):
        rep16[x, 32 * x:32 * x + 16] = 1.0
    rep32 = np.zeros((3, 96), np.float32)
    for x in range(3):
        rep32[x, 32 * x:32 * x + 32] = 1.0
    rep144a = np.zeros((9, 128), np.float32)
    rep144b = np.zeros((9, 128), np.float32)
    rep144c = np.zeros((9, 32), np.float32)
    for ij in range(4):
        rep144a[ij, 32 * ij:32 * ij + 16] = 1.0
    for ij in range(4, 8):
        rep144b[ij, 32 * (ij - 4):32 * (ij - 4) + 16] = 1.0
    rep144c[8, 0:16] = 1.0
    repv32 = np.zeros((32, 96), np.float32)
    for x in range(3):
        repv32[:, 32 * x:32 * x + 32] = np.eye(32, dtype=np.float32)
    khalf4 = np.tile((0.5 * np.arange(8, dtype=np.float32))[None, :], (128, 2))
    return dict(khalf4=_f32(khalf4), w2sa=w2sa, w2sb=w2sb, wavd=wavd, wx1=wx1,
                wv1=wv1, wd1=wd1, woa=woa, wov=wov, wod=wod, rep16=rep16,
                rep32=rep32, rep144a=rep144a, rep144b=rep144b,
                rep144c=rep144c, repv32=repv32)


def _run_spmd(nc, in_maps):
    from concourse.bass_utils import run_bass_kernel_spmd
    return run_bass_kernel_spmd(nc, in_maps, list(range(NC))).results


def kernel(**inputs):
    pos = _f32(inputs["pos"])
    src = np.asarray(inputs["src"]).astype(np.int64)
    dst = np.asarray(inputs["dst"]).astype(np.int64)

    # ---- launch A: |r|^2 for all edges
    if "A" not in _CACHE:
        _CACHE["A"] = _build_a()
    nca = _CACHE["A"]
    pospad = np.zeros((N_NODES, 64), np.float32)
    pospad[:, :3] = pos
    in_maps = []
    for c in range(NC):
        sl = slice(c * EPC, (c + 1) * EPC)
        in_maps.append({"pospad": pospad, "gsrc": _wrap_idx(src[sl]),
                        "gdst": _wrap_idx(dst[sl])})
    res_a = _run_spmd(nca, in_maps)
    mask = np.empty(N_EDGES, bool)
    for c in range(NC):
        xs = res_a[c]["xsq"]            # [128, GRP]
        flat = xs.T.reshape(-1)[:EPC]   # edge e at [e%128, e//128]
        mask[c * EPC:(c + 1) * EPC] = flat < 4.0

    # ---- launch B: message passing for survivors
    T = T_DEF
    while True:
        try:
            per_core = _host_prep_b(inputs, mask, T)
            break
        except RuntimeError:
            T += 8
    if ("B", T) not in _CACHE:
        _CACHE[("B", T)] = _build_b(T)
    ncb = _CACHE[("B", T)]

    fake = (pos[0] + 10.0) % 20.0
    pos3 = _f32(np.concatenate([pos, fake[None, :]], 0))
    posx = np.zeros((N_NODES + 1, 307), np.float32)
    posx[:, 0:3] = pos3
    posx[:N_NODES, 3:67] = np.asarray(inputs["x_a"], np.float32)
    posx[:N_NODES, 67:163] = np.asarray(inputs["x_v"],
                                        np.float32).reshape(N_NODES, 96)
    posx[:N_NODES, 163:307] = np.asarray(inputs["x_d"],
                                         np.float32).reshape(N_NODES, 144)
    wts = _weights_b(inputs)
    in_maps = []
    for c in range(NC):
        srcg, dstg, elocf, _tm = per_core[c]
        m = {"pos3": pos3, "posx": posx,
             "srcg": srcg, "dstg": dstg, "elocf": elocf}
        m.update(wts)
        in_maps.append(m)
    res_b = _run_spmd(ncb, in_maps)

    out_a = np.zeros((N_NODES, 64), np.float32)
    out_v = np.zeros((N_NODES, 96), np.float32)
    out_d = np.zeros((N_NODES, 144), np.float32)
    for c in range(NC):
        _s, _d, _e, tile_map = per_core[c]
        acc = np.zeros((T_DEF * 128, OUTC), np.float32)
        op = res_b[c]["outp"]
        for t, w in enumerate(tile_map):
            acc[128 * w:128 * (w + 1)] += op[128 * t:128 * (t + 1)]
        rows = slice(c * NPC, (c + 1) * NPC)
        out_a[rows] = acc[:NPC, 0:64]
        out_v[rows] = acc[:NPC, 64:160]
        out_d[rows] = acc[:NPC, 160:304]
    return (out_a, out_v.reshape(N_NODES, 32, 3),
            out_d.reshape(N_NODES, 16, 3, 3))
